# revision 7
# baseline (speedup 1.0000x reference)
"""Trainium2 Bass kernel for nn_ChamferLoss (retrieval_knn).

Computes, for preds/targ of shape [64, 32, 32772] (fp32):
  - action MSE losses over the first 4 channels
  - Chamfer loss over the remaining 32768 channels viewed as 256 points x 128 dims
Returns (action_loss + chamfer_loss, a0_loss) as fp32 scalars.

Strategy (pure data-parallel over batch, 8 NeuronCores):
  Each core handles 8 batches = 256 (b,h) groups. Per group, with
  x = targ points [256,128], y = pred points [256,128] (bf16, host-transposed
  to [d, n] layout so the PE contracts over d directly):

    S[i,j] = x_i . y_j - 0.5||y_j||^2 - 0.5||x_i||^2  = -P[i,j]/2
    loss_2 = sum_i min_j P = -2 sum_i max_j S     (DVE free-axis reduce)
    loss_1 = sum_j min_i P = -2 sum_j max_i S     (PE transpose + DVE reduce)

  PE: 2 main matmuls (i-chunks) + 2 rank-1 fold matmuls add -0.5||y_j||^2
  ACT: PSUM->SBUF copy with per-partition bias -0.5||x_i||^2
  GPSIMD: elementwise max of the two i-chunks (for the i-direction max)
  PE: transpose of the chunk-max, DVE: both final reduces.
  Host: final scalar sums over the tiny per-core [128, 512] outputs.
"""

import os
import sys

import numpy as np

for _p in ("/root/.axon_site", "/root/.axon_site/_ro/trn_rl_repo",
           "/root/.axon_site/_ro/pypackages", "/opt/trn_rl_repo"):
    if os.path.isdir(_p) and _p not in sys.path:
        sys.path.append(_p)

import ml_dtypes

import concourse.bacc as bacc
import concourse.mybir as mybir
import concourse.tile as tile
from concourse.bass_utils import run_bass_kernel_spmd
from concourse.masks import make_identity

BF16 = ml_dtypes.bfloat16

B, H, T = 64, 32, 32772
AD, OD = 4, 128          # action dim, obs dim
NPTS = 256               # points per group (256 x 128 = 32768 obs channels)
D = 128                  # point dim
NCORES = 8
BLOC = B // NCORES       # batches per core
G_FULL = BLOC * H        # groups per core = 256


def build_program(G=G_FULL, bufs=6):
    """Build the per-core Bass program (same program runs SPMD on all cores)."""
    from contextlib import ExitStack

    nc = bacc.Bacc("TRN2", target_bir_lowering=False)
    f32 = mybir.dt.float32
    bf16 = mybir.dt.bfloat16

    xt = nc.dram_tensor("xt", [G, D, NPTS], bf16, kind="ExternalInput")
    yt = nc.dram_tensor("yt", [G, D, NPTS], bf16, kind="ExternalInput")
    y2d = nc.dram_tensor("y2d", [G, 2 * NPTS], bf16, kind="ExternalInput")
    x2b = nc.dram_tensor("x2b", [D, 2 * G], f32, kind="ExternalInput")
    actp = nc.dram_tensor("actp", [128, 8], f32, kind="ExternalInput")
    actt = nc.dram_tensor("actt", [128, 8], f32, kind="ExternalInput")
    r2o = nc.dram_tensor("r2o", [128, 2 * G], f32, kind="ExternalOutput")
    r1o = nc.dram_tensor("r1o", [128, 2 * G], f32, kind="ExternalOutput")
    acto = nc.dram_tensor("acto", [128, 2], f32, kind="ExternalOutput")

    ID = mybir.ActivationFunctionType.Identity
    MAX = mybir.AluOpType.max
    ADDOP = mybir.AluOpType.add
    X = mybir.AxisListType.X

    with ExitStack() as ctx:
        tc = ctx.enter_context(tile.TileContext(nc))
        singles = ctx.enter_context(tc.tile_pool(name="singles", bufs=1))
        loads = ctx.enter_context(tc.tile_pool(name="loads", bufs=bufs))
        y2pool = ctx.enter_context(tc.tile_pool(name="y2pool", bufs=bufs))
        spool = ctx.enter_context(tc.tile_pool(name="spool", bufs=4))
        sbarpool = ctx.enter_context(tc.tile_pool(name="sbarpool", bufs=4))
        psum_acc = ctx.enter_context(tc.tile_pool(name="psum_acc", bufs=3, space="PSUM"))
        psum_t = ctx.enter_context(tc.tile_pool(name="psum_t", bufs=3, space="PSUM"))

        ident = singles.tile([128, 128], bf16)
        make_identity(nc, ident[:])
        ones = singles.tile([1, 128], bf16)
        nc.vector.memset(ones[:], 1.0)
        x2b_t = singles.tile([D, 2 * G], f32)
        nc.sync.dma_start(x2b_t[:], x2b[:])
        r2acc = singles.tile([128, 2 * G], f32)
        r1acc = singles.tile([128, 2 * G], f32)

        # action losses (tiny, once)
        ap_t = singles.tile([128, 8], f32)
        nc.sync.dma_start(ap_t[:], actp[:])
        at_t = singles.tile([128, 8], f32)
        nc.sync.dma_start(at_t[:], actt[:])
        d_t = singles.tile([128, 8], f32)
        nc.vector.tensor_sub(d_t[:], ap_t[:], at_t[:])
        sq_t = singles.tile([128, 8], f32)
        nc.vector.tensor_mul(sq_t[:], d_t[:], d_t[:])
        aco_t = singles.tile([128, 2], f32)
        nc.vector.tensor_reduce(
            aco_t[:], sq_t[:].rearrange("p (c k) -> p c k", c=2), axis=X, op=ADDOP
        )
        nc.sync.dma_start(acto[:], aco_t[:])

        for g in range(G):
            xt_t = loads.tile([D, NPTS], bf16, tag="xt")
            nc.sync.dma_start(xt_t[:], xt[g])
            yt_t = loads.tile([D, NPTS], bf16, tag="yt")
            nc.sync.dma_start(yt_t[:], yt[g])
            y2_t = y2pool.tile([1, 2 * NPTS], bf16)
            nc.sync.dma_start(y2_t[:], y2d[g : g + 1, :])

            acc = psum_acc.tile([128, 512], f32)
            # Fold first (start=True over the whole bank), then the two main
            # matmuls accumulate: keeps ONE accumulation group per PSUM bank
            # (a second start=True in the same bank clears the whole bank's
            # has_written bits and silently drops the earlier partial).
            nc.tensor.matmul(acc[:, 0:512], lhsT=ones[:], rhs=y2_t[:],
                             start=True, stop=False)
            nc.tensor.matmul(acc[:, 0:256], lhsT=xt_t[:, 0:128], rhs=yt_t[:],
                             start=False, stop=False)
            nc.tensor.matmul(acc[:, 256:512], lhsT=xt_t[:, 128:256], rhs=yt_t[:],
                             start=False, stop=True)

            s_t = spool.tile([128, 512], bf16)
            nc.scalar.activation(s_t[:, 0:256], acc[:, 0:256], func=ID,
                                 bias=x2b_t[:, g : g + 1], scale=1.0)
            nc.scalar.activation(s_t[:, 256:512], acc[:, 256:512], func=ID,
                                 bias=x2b_t[:, G + g : G + g + 1], scale=1.0)

            # loss_2 direction: max over j (free axis), both i-chunks at once
            nc.vector.tensor_reduce(
                r2acc[:, 2 * g : 2 * g + 2],
                s_t[:].rearrange("p (c j) -> p c j", c=2),
                axis=X, op=MAX,
            )

            # loss_1 direction: combine i-chunks, transpose, reduce over i
            sbar = sbarpool.tile([128, NPTS], bf16)
            nc.vector.tensor_max(sbar[:], s_t[:, 0:256], s_t[:, 256:512])
            stp = psum_t.tile([128, 256], bf16)
            nc.tensor.transpose(stp[:, 0:128], sbar[:, 0:128], ident[:])
            nc.tensor.transpose(stp[:, 128:256], sbar[:, 128:256], ident[:])
            nc.vector.tensor_reduce(
                r1acc[:, 2 * g : 2 * g + 2],
                stp[:].rearrange("p (c i) -> p c i", c=2),
                axis=X, op=MAX,
            )

        nc.sync.dma_start(r2o[:], r2acc[:])
        nc.sync.dma_start(r1o[:], r1acc[:])

    nc.finalize()
    return nc


def preprocess(preds, targ, ncores=NCORES):
    """Host-side: slice/transposes/norms -> per-core input maps."""
    preds = np.asarray(preds)
    targ = np.asarray(targ)
    assert preds.shape == (B, H, T) and preds.dtype == np.float32

    obs_p = preds[:, :, AD:].reshape(B, H, NPTS, D)
    obs_t = targ[:, :, AD:].reshape(B, H, NPTS, D)
    p_bf = obs_p.astype(BF16)           # y (preds)
    t_bf = obs_t.astype(BF16)           # x (targ / gts)
    # norms computed from the bf16-rounded values, in fp32
    y2 = np.square(p_bf.astype(np.float32)).sum(-1)   # [B, H, 256]
    x2 = np.square(t_bf.astype(np.float32)).sum(-1)

    act_p = preds[:, :, :AD].reshape(B * H, AD)
    act_t = targ[:, :, :AD].reshape(B * H, AD)

    bloc = B // ncores
    g = bloc * H
    in_maps = []
    for c in range(ncores):
        sl = slice(bloc * c, bloc * (c + 1))
        xt_c = np.ascontiguousarray(
            t_bf[sl].transpose(0, 1, 3, 2).reshape(g, D, NPTS))
        yt_c = np.ascontiguousarray(
            p_bf[sl].transpose(0, 1, 3, 2).reshape(g, D, NPTS))
        y2_c = (-0.5 * y2[sl].reshape(g, NPTS)).astype(BF16)
        y2d_c = np.concatenate([y2_c, y2_c], axis=1)          # [g, 512]
        x2_c = (-0.5 * x2[sl].reshape(g, NPTS)).astype(np.float32)
        x2b_c = np.ascontiguousarray(
            x2_c.reshape(g, 2, 128).transpose(2, 1, 0).reshape(128, 2 * g))
        rows = slice(g * c, g * (c + 1))
        ap_c = np.ascontiguousarray(
            act_p[rows].reshape(2, 128, AD).transpose(1, 0, 2).reshape(128, 8))
        at_c = np.ascontiguousarray(
            act_t[rows].reshape(2, 128, AD).transpose(1, 0, 2).reshape(128, 8))
        in_maps.append(dict(xt=xt_c, yt=yt_c, y2d=y2d_c, x2b=x2b_c,
                            actp=ap_c, actt=at_c))
    return in_maps


def postprocess(results):
    """Host-side: combine per-core partial outputs into the two loss scalars."""
    loss12 = 0.0
    mse = np.zeros((B, H), dtype=np.float64)
    bloc = B // len(results)
    for c, r in enumerate(results):
        loss12 += -2.0 * (r["r2o"].astype(np.float64).sum()
                          + r["r1o"].astype(np.float64).sum())
        aco = r["acto"].astype(np.float64)            # [128, 2]
        rows = aco.T.reshape(2 * 128) / AD            # row = c2*128 + p
        mse[bloc * c : bloc * (c + 1)] = rows.reshape(bloc, H)
    chamfer = loss12 / (B * H)
    a0_loss = mse[:, 0].mean()
    w = np.ones(H, dtype=np.float64)
    w[0] = 10.0
    action_loss = (mse * w[None, :]).mean()
    return (np.float32(action_loss + chamfer), np.float32(a0_loss))


_NC_CACHE = {}


def _get_program():
    if "nc" not in _NC_CACHE:
        _NC_CACHE["nc"] = build_program()
    return _NC_CACHE["nc"]


def kernel(preds, targ):
    nc = _get_program()
    in_maps = preprocess(preds, targ)
    results = run_bass_kernel_spmd(nc, in_maps, core_ids=list(range(NCORES))).results
    return postprocess(results)


# revision 28
# speedup vs baseline: 4.1614x; 4.1614x over previous
"""Trainium2 Bass kernel for nn_ChamferLoss (retrieval_knn).

Computes, for preds/targ of shape [64, 32, 32772] (fp32):
  - action MSE losses over the first 4 channels
  - Chamfer loss over the remaining 32768 channels viewed as 256 points x 128 dims
Returns (action_loss + chamfer_loss, a0_loss) as fp32 scalars.

Strategy (pure data-parallel over batch, 8 NeuronCores):
  Each core handles 8 batches = 256 (b,h) groups. Per group, with
  x = targ points [256,128], y = pred points [256,128] (bf16, host-transposed
  to [d, n] layout so the PE contracts over d directly):

    S[i,j] = x_i . y_j - 0.5||y_j||^2 - 0.5||x_i||^2  = -P[i,j]/2
    loss_2 = sum_i min_j P = -2 sum_i max_j S     (DVE free-axis reduce)
    loss_1 = sum_j min_i P = -2 sum_j max_i S     (PE transpose + DVE reduce)

  PE: 2 main matmuls (i-chunks) + 2 rank-1 fold matmuls add -0.5||y_j||^2
  ACT: PSUM->SBUF copy with per-partition bias -0.5||x_i||^2
  GPSIMD: elementwise max of the two i-chunks (for the i-direction max)
  PE: transpose of the chunk-max, DVE: both final reduces.
  Host: final scalar sums over the tiny per-core [128, 512] outputs.
"""

import os
import sys

import numpy as np

for _p in ("/root/.axon_site", "/root/.axon_site/_ro/trn_rl_repo",
           "/root/.axon_site/_ro/pypackages", "/opt/trn_rl_repo"):
    if os.path.isdir(_p) and _p not in sys.path:
        sys.path.append(_p)

import ml_dtypes

import concourse.bacc as bacc
import concourse.mybir as mybir
import concourse.tile as tile
from concourse.bass_utils import run_bass_kernel_spmd
from concourse.masks import make_identity

BF16 = ml_dtypes.bfloat16

B, H, T = 64, 32, 32772
AD, OD = 4, 128          # action dim, obs dim
NPTS = 256               # points per group (256 x 128 = 32768 obs channels)
D = 128                  # point dim
NCORES = 8
BLOC = B // NCORES       # batches per core
G_FULL = BLOC * H        # groups per core = 256


def build_program(G=G_FULL, bufs=2, blk=32, reps=1, stage=5):
    """Build the per-core Bass program (same program runs SPMD on all cores).

    xt/yt are d-major ([D, G, NPTS]) so one DMA per blk-group block reads
    blk*512 contiguous bytes per partition (few descriptors, deep transfers).
    """
    from contextlib import ExitStack

    nc = bacc.Bacc("TRN2", target_bir_lowering=False)
    f32 = mybir.dt.float32
    bf16 = mybir.dt.bfloat16
    blk = min(blk, G)
    assert G % blk == 0

    xt = nc.dram_tensor("xt", [D, G, NPTS], bf16, kind="ExternalInput")
    yt = nc.dram_tensor("yt", [D, G, NPTS], bf16, kind="ExternalInput")
    y2d = nc.dram_tensor("y2d", [G, 2 * NPTS], bf16, kind="ExternalInput")
    x2b = nc.dram_tensor("x2b", [D, 2 * G], f32, kind="ExternalInput")
    actp = nc.dram_tensor("actp", [128, 8], f32, kind="ExternalInput")
    actt = nc.dram_tensor("actt", [128, 8], f32, kind="ExternalInput")
    r2o = nc.dram_tensor("r2o", [128, 2 * G], f32, kind="ExternalOutput")
    r1o = nc.dram_tensor("r1o", [128, 2 * G], f32, kind="ExternalOutput")
    acto = nc.dram_tensor("acto", [128, 2], f32, kind="ExternalOutput")

    ID = mybir.ActivationFunctionType.Identity
    MAX = mybir.AluOpType.max
    ADDOP = mybir.AluOpType.add
    X = mybir.AxisListType.X
    FMIN = float(np.finfo(np.float32).min)

    with ExitStack() as ctx:
        tc = ctx.enter_context(tile.TileContext(nc))
        singles = ctx.enter_context(tc.tile_pool(name="singles", bufs=1))
        loads = ctx.enter_context(tc.tile_pool(name="loads", bufs=bufs))
        y2pool = ctx.enter_context(tc.tile_pool(name="y2pool", bufs=bufs))
        del bufs
        spool = ctx.enter_context(tc.tile_pool(name="spool", bufs=4))
        sbarpool = ctx.enter_context(tc.tile_pool(name="sbarpool", bufs=5))
        scrpool = ctx.enter_context(tc.tile_pool(name="scrpool", bufs=2))
        psum_acc = ctx.enter_context(tc.tile_pool(name="psum_acc", bufs=4, space="PSUM"))
        psum_t = ctx.enter_context(tc.tile_pool(name="psum_t", bufs=4, space="PSUM"))

        ident = singles.tile([128, 128], bf16)
        make_identity(nc, ident[:])
        ones = singles.tile([1, 128], bf16)
        nc.vector.memset(ones[:], 1.0)
        x2b_t = singles.tile([D, 2 * G], f32)
        nc.sync.dma_start(x2b_t[:], x2b[:])
        r2acc = singles.tile([128, 2 * G], f32)
        r1acc = singles.tile([128, 2 * G], f32)
        nc.gpsimd.memset(r2acc[:], 0.0)
        nc.gpsimd.memset(r1acc[:], 0.0)

        # action losses (tiny, once)
        ap_t = singles.tile([128, 8], f32)
        nc.sync.dma_start(ap_t[:], actp[:])
        at_t = singles.tile([128, 8], f32)
        nc.sync.dma_start(at_t[:], actt[:])
        d_t = singles.tile([128, 8], f32)
        nc.vector.tensor_sub(d_t[:], ap_t[:], at_t[:])
        sq_t = singles.tile([128, 8], f32)
        nc.vector.tensor_mul(sq_t[:], d_t[:], d_t[:])
        aco_t = singles.tile([128, 2], f32)
        nc.vector.tensor_reduce(
            aco_t[:], sq_t[:].rearrange("p (c k) -> p c k", c=2), axis=X, op=ADDOP
        )
        nc.sync.dma_start(acto[:], aco_t[:])

        SKEW = 2
        pending = []

        def drain_one():
            g, sbar = pending.pop(0)
            stp = psum_t.tile([128, 256], bf16, tag="stp")
            nc.tensor.transpose(stp[:, 0:128], sbar[:, 0:128], ident[:])
            nc.tensor.transpose(stp[:, 128:256], sbar[:, 128:256], ident[:])
            nc.vector.tensor_reduce(
                r1acc[:, 2 * g : 2 * g + 2],
                stp[:].rearrange("p (c i) -> p c i", c=2),
                axis=X, op=MAX,
            )

        for b in [bb for _ in range(reps) for bb in range(G // blk)]:
            xts = loads.tile([D, blk, NPTS], bf16, tag="xts")
            nc.sync.dma_start(xts[:], xt[:, b * blk : (b + 1) * blk, :])
            yts = loads.tile([D, blk, NPTS], bf16, tag="yts")
            nc.sync.dma_start(yts[:], yt[:, b * blk : (b + 1) * blk, :])
            y2blk = y2pool.tile([1, blk, 2 * NPTS], bf16)
            nc.sync.dma_start(
                y2blk[:],
                y2d[b * blk : (b + 1) * blk, :].rearrange("g n -> (g n)")[None, :],
            )

            # per-group compute over the resident block
            for gi in range(blk):
                g = b * blk + gi
                xt_t = xts[:, gi, :]
                yt_t = yts[:, gi, :]
                y2_t = y2blk[:, gi, :]

                if stage < 1:
                    continue
                acc = psum_acc.tile([128, 512], f32)
                # Fold first (start=True over the whole bank), then the two
                # main matmuls accumulate: keeps ONE accumulation group per
                # PSUM bank (a second start=True in the same bank clears the
                # whole bank's has_written bits, dropping earlier partials).
                nc.tensor.matmul(acc[:, 0:512], lhsT=ones[:], rhs=y2_t,
                                 start=True, stop=False)
                nc.tensor.matmul(acc[:, 0:256], lhsT=xt_t[:, 0:128], rhs=yt_t,
                                 start=False, stop=False)
                nc.tensor.matmul(acc[:, 256:512], lhsT=xt_t[:, 128:256], rhs=yt_t,
                                 start=False, stop=True)

                if stage < 2:
                    continue
                s_t = spool.tile([128, 512], bf16)
                nc.scalar.activation(s_t[:, 0:256], acc[:, 0:256], func=ID,
                                     bias=x2b_t[:, g : g + 1], scale=1.0)
                nc.scalar.activation(s_t[:, 256:512], acc[:, 256:512], func=ID,
                                     bias=x2b_t[:, G + g : G + g + 1], scale=1.0)

                if stage < 3:
                    continue
                # loss_2 direction: max over j (free axis), both i-chunks
                nc.vector.tensor_reduce(
                    r2acc[:, 2 * g : 2 * g + 2],
                    s_t[:].rearrange("p (c j) -> p c j", c=2),
                    axis=X, op=MAX,
                )

                if stage < 4:
                    continue
                # loss_1 first half: combine i-chunks (DVE 2x bf16)
                sbar = sbarpool.tile([128, NPTS], bf16)
                nc.vector.tensor_max(sbar[:], s_t[:, 0:256], s_t[:, 256:512])
                if stage < 5:
                    continue
                pending.append((g, sbar))

                # loss_1 second half for group g-SKEW: transpose + reduce.
                # Deferring keeps the PE's in-order queue from stalling on
                # sbar (which depends on ACT+DVE for the current group).
                if len(pending) > SKEW:
                    drain_one()

        while pending:
            drain_one()

        nc.sync.dma_start(r2o[:], r2acc[:])
        nc.sync.dma_start(r1o[:], r1acc[:])

    nc.finalize()
    return nc


def preprocess(preds, targ, ncores=NCORES):
    """Host-side: slice/transposes/norms -> per-core input maps."""
    preds = np.asarray(preds)
    targ = np.asarray(targ)
    assert preds.shape == (B, H, T) and preds.dtype == np.float32

    obs_p = preds[:, :, AD:].reshape(B, H, NPTS, D)
    obs_t = targ[:, :, AD:].reshape(B, H, NPTS, D)
    p_bf = obs_p.astype(BF16)           # y (preds)
    t_bf = obs_t.astype(BF16)           # x (targ / gts)
    # norms computed from the bf16-rounded values, in fp32
    y2 = np.square(p_bf.astype(np.float32)).sum(-1)   # [B, H, 256]
    x2 = np.square(t_bf.astype(np.float32)).sum(-1)

    act_p = preds[:, :, :AD].reshape(B * H, AD)
    act_t = targ[:, :, :AD].reshape(B * H, AD)

    bloc = B // ncores
    g = bloc * H
    in_maps = []
    for c in range(ncores):
        sl = slice(bloc * c, bloc * (c + 1))
        xt_c = np.ascontiguousarray(
            t_bf[sl].transpose(3, 0, 1, 2).reshape(D, g, NPTS))
        yt_c = np.ascontiguousarray(
            p_bf[sl].transpose(3, 0, 1, 2).reshape(D, g, NPTS))
        y2_c = (-0.5 * y2[sl].reshape(g, NPTS)).astype(BF16)
        y2d_c = np.concatenate([y2_c, y2_c], axis=1)          # [g, 512]
        x2_c = (-0.5 * x2[sl].reshape(g, NPTS)).astype(np.float32)
        x2b_c = np.ascontiguousarray(
            x2_c.reshape(g, 2, 128).transpose(2, 1, 0).reshape(128, 2 * g))
        rows = slice(g * c, g * (c + 1))
        ap_c = np.ascontiguousarray(
            act_p[rows].reshape(2, 128, AD).transpose(1, 0, 2).reshape(128, 8))
        at_c = np.ascontiguousarray(
            act_t[rows].reshape(2, 128, AD).transpose(1, 0, 2).reshape(128, 8))
        in_maps.append(dict(xt=xt_c, yt=yt_c, y2d=y2d_c, x2b=x2b_c,
                            actp=ap_c, actt=at_c))
    return in_maps


def postprocess(results):
    """Host-side: combine per-core partial outputs into the two loss scalars."""
    loss12 = 0.0
    mse = np.zeros((B, H), dtype=np.float64)
    bloc = B // len(results)
    for c, r in enumerate(results):
        loss12 += -2.0 * (r["r2o"].astype(np.float64).sum()
                          + r["r1o"].astype(np.float64).sum())
        aco = r["acto"].astype(np.float64)            # [128, 2]
        rows = aco.T.reshape(2 * 128) / AD            # row = c2*128 + p
        mse[bloc * c : bloc * (c + 1)] = rows.reshape(bloc, H)
    chamfer = loss12 / (B * H)
    a0_loss = mse[:, 0].mean()
    w = np.ones(H, dtype=np.float64)
    w[0] = 10.0
    action_loss = (mse * w[None, :]).mean()
    return (np.float32(action_loss + chamfer), np.float32(a0_loss))


_NC_CACHE = {}


def _get_program():
    if "nc" not in _NC_CACHE:
        _NC_CACHE["nc"] = build_program()
    return _NC_CACHE["nc"]


def kernel(preds, targ):
    nc = _get_program()
    in_maps = preprocess(preds, targ)
    results = run_bass_kernel_spmd(nc, in_maps, core_ids=list(range(NCORES))).results
    return postprocess(results)


# revision 32
# speedup vs baseline: 5.0849x; 1.2219x over previous
"""Trainium2 Bass kernel for nn_ChamferLoss (retrieval_knn).

Computes, for preds/targ of shape [64, 32, 32772] (fp32):
  - action MSE losses over the first 4 channels
  - Chamfer loss over the remaining 32768 channels viewed as 256 points x 128 dims
Returns (action_loss + chamfer_loss, a0_loss) as fp32 scalars.

Strategy (pure data-parallel over batch, 8 NeuronCores):
  Each core handles 8 batches = 256 (b,h) groups. Per group, with
  x = targ points [256,128], y = pred points [256,128] (bf16, host-transposed
  to [d, n] layout so the PE contracts over d directly):

    S[i,j] = x_i . y_j - 0.5||y_j||^2 - 0.5||x_i||^2  = -P[i,j]/2
    loss_2 = sum_i min_j P = -2 sum_i max_j S     (DVE free-axis reduce)
    loss_1 = sum_j min_i P = -2 sum_j max_i S     (PE transpose + DVE reduce)

  PE: rank-1 fold matmul adds -0.5||y_j||^2, then 2 main matmuls (i-chunks)
      accumulate x.y into the same PSUM bank.
  ACT: PSUM->SBUF bf16 copy with per-partition bias -0.5||x_i||^2  -> S.
  DVE: free-axis max-reduce of S (loss_2); 2x-mode bf16 pairwise max of the
      two i-chunks; after a PE transpose of that, free-axis max-reduce (loss_1).
  Host: final scalar sums over the tiny per-core [128, 512] outputs.
"""

import os
import sys

import numpy as np

for _p in ("/root/.axon_site", "/root/.axon_site/_ro/trn_rl_repo",
           "/root/.axon_site/_ro/pypackages", "/opt/trn_rl_repo"):
    if os.path.isdir(_p) and _p not in sys.path:
        sys.path.append(_p)

import ml_dtypes

import concourse.bacc as bacc
import concourse.mybir as mybir
import concourse.tile as tile
from concourse.bass_utils import run_bass_kernel_spmd
from concourse.masks import make_identity

BF16 = ml_dtypes.bfloat16

B, H, T = 64, 32, 32772
AD, OD = 4, 128          # action dim, obs dim
NPTS = 256               # points per group (256 x 128 = 32768 obs channels)
D = 128                  # point dim
NCORES = 8
BLOC = B // NCORES       # batches per core
G_FULL = BLOC * H        # groups per core = 256


def build_program(G=G_FULL, bufs=2, blk=32, reps=1, stage=5):
    """Build the per-core Bass program (same program runs SPMD on all cores).

    xt/yt are d-major ([D, G, NPTS]) so one DMA per blk-group block reads
    blk*512 contiguous bytes per partition (few descriptors, deep transfers).
    """
    from contextlib import ExitStack

    nc = bacc.Bacc("TRN2", target_bir_lowering=False)
    f32 = mybir.dt.float32
    bf16 = mybir.dt.bfloat16
    blk = min(blk, G)
    assert G % blk == 0

    xt = nc.dram_tensor("xt", [D, G, NPTS], bf16, kind="ExternalInput")
    yt = nc.dram_tensor("yt", [D, G, NPTS], bf16, kind="ExternalInput")
    y2d = nc.dram_tensor("y2d", [G, 2 * NPTS], bf16, kind="ExternalInput")
    x2b = nc.dram_tensor("x2b", [D, 2 * G], f32, kind="ExternalInput")
    actp = nc.dram_tensor("actp", [128, 8], f32, kind="ExternalInput")
    actt = nc.dram_tensor("actt", [128, 8], f32, kind="ExternalInput")
    r2o = nc.dram_tensor("r2o", [128, 2 * G], f32, kind="ExternalOutput")
    r1o = nc.dram_tensor("r1o", [128, 2 * G], f32, kind="ExternalOutput")
    acto = nc.dram_tensor("acto", [128, 2], f32, kind="ExternalOutput")

    ID = mybir.ActivationFunctionType.Identity
    MAX = mybir.AluOpType.max
    ADDOP = mybir.AluOpType.add
    X = mybir.AxisListType.X

    with ExitStack() as ctx:
        tc = ctx.enter_context(tile.TileContext(nc))
        singles = ctx.enter_context(tc.tile_pool(name="singles", bufs=1))
        loads = ctx.enter_context(tc.tile_pool(name="loads", bufs=bufs))
        y2pool = ctx.enter_context(tc.tile_pool(name="y2pool", bufs=bufs))
        del bufs
        spool = ctx.enter_context(tc.tile_pool(name="spool", bufs=4))
        sbarpool = ctx.enter_context(tc.tile_pool(name="sbarpool", bufs=5))
        psum_acc = ctx.enter_context(tc.tile_pool(name="psum_acc", bufs=4, space="PSUM"))
        psum_t = ctx.enter_context(tc.tile_pool(name="psum_t", bufs=4, space="PSUM"))

        ident = singles.tile([128, 128], bf16)
        make_identity(nc, ident[:])
        ones = singles.tile([1, 128], bf16)
        nc.vector.memset(ones[:], 1.0)
        x2b_t = singles.tile([D, 2 * G], f32)
        nc.sync.dma_start(x2b_t[:], x2b[:])
        r2acc = singles.tile([128, 2 * G], f32)
        r1acc = singles.tile([128, 2 * G], f32)
        nc.gpsimd.memset(r2acc[:], 0.0)
        nc.gpsimd.memset(r1acc[:], 0.0)

        # action losses (tiny, once)
        ap_t = singles.tile([128, 8], f32)
        nc.sync.dma_start(ap_t[:], actp[:])
        at_t = singles.tile([128, 8], f32)
        nc.sync.dma_start(at_t[:], actt[:])
        d_t = singles.tile([128, 8], f32)
        nc.vector.tensor_sub(d_t[:], ap_t[:], at_t[:])
        sq_t = singles.tile([128, 8], f32)
        nc.vector.tensor_mul(sq_t[:], d_t[:], d_t[:])
        aco_t = singles.tile([128, 2], f32)
        nc.vector.tensor_reduce(
            aco_t[:], sq_t[:].rearrange("p (c k) -> p c k", c=2), axis=X, op=ADDOP
        )
        nc.sync.dma_start(acto[:], aco_t[:])

        SKEW = 2
        pending = []

        def drain_one():
            g, sbar = pending.pop(0)
            stp = psum_t.tile([128, 256], bf16, tag="stp")
            nc.tensor.transpose(stp[:, 0:128], sbar[:, 0:128], ident[:])
            nc.tensor.transpose(stp[:, 128:256], sbar[:, 128:256], ident[:])
            nc.vector.tensor_reduce(
                r1acc[:, 2 * g : 2 * g + 2],
                stp[:].rearrange("p (c i) -> p c i", c=2),
                axis=X, op=MAX,
            )

        for b in [bb for _ in range(reps) for bb in range(G // blk)]:
            xts = loads.tile([D, blk, NPTS], bf16, tag="xts")
            nc.sync.dma_start(xts[:], xt[:, b * blk : (b + 1) * blk, :])
            yts = loads.tile([D, blk, NPTS], bf16, tag="yts")
            nc.sync.dma_start(yts[:], yt[:, b * blk : (b + 1) * blk, :])
            y2blk = y2pool.tile([1, blk, 2 * NPTS], bf16)
            nc.sync.dma_start(
                y2blk[:],
                y2d[b * blk : (b + 1) * blk, :].rearrange("g n -> (g n)")[None, :],
            )

            # per-group compute over the resident block
            for gi in range(blk):
                g = b * blk + gi
                xt_t = xts[:, gi, :]
                yt_t = yts[:, gi, :]
                y2_t = y2blk[:, gi, :]

                if stage < 1:
                    continue
                acc = psum_acc.tile([128, 512], f32)
                # Fold first (start=True over the whole bank), then the two
                # main matmuls accumulate: keeps ONE accumulation group per
                # PSUM bank (a second start=True in the same bank clears the
                # whole bank's has_written bits, dropping earlier partials).
                nc.tensor.matmul(acc[:, 0:512], lhsT=ones[:], rhs=y2_t,
                                 start=True, stop=False)
                nc.tensor.matmul(acc[:, 0:256], lhsT=xt_t[:, 0:128], rhs=yt_t,
                                 start=False, stop=False)
                nc.tensor.matmul(acc[:, 256:512], lhsT=xt_t[:, 128:256], rhs=yt_t,
                                 start=False, stop=True)

                if stage < 2:
                    continue
                s_t = spool.tile([128, 512], bf16)
                nc.scalar.activation(s_t[:, 0:256], acc[:, 0:256], func=ID,
                                     bias=x2b_t[:, g : g + 1], scale=1.0)
                nc.scalar.activation(s_t[:, 256:512], acc[:, 256:512], func=ID,
                                     bias=x2b_t[:, G + g : G + g + 1], scale=1.0)

                if stage < 3:
                    continue
                # loss_2 direction: max over j (free axis), both i-chunks
                nc.vector.tensor_reduce(
                    r2acc[:, 2 * g : 2 * g + 2],
                    s_t[:].rearrange("p (c j) -> p c j", c=2),
                    axis=X, op=MAX,
                )

                if stage < 4:
                    continue
                # loss_1 first half: combine i-chunks (DVE 2x bf16)
                sbar = sbarpool.tile([128, NPTS], bf16)
                nc.vector.tensor_max(sbar[:], s_t[:, 0:256], s_t[:, 256:512])
                if stage < 5:
                    continue
                pending.append((g, sbar))

                # loss_1 second half for group g-SKEW: transpose + reduce.
                # Deferring keeps the PE's in-order queue from stalling on
                # sbar (which depends on ACT+DVE for the current group).
                if len(pending) > SKEW:
                    drain_one()

        while pending:
            drain_one()

        nc.sync.dma_start(r2o[:], r2acc[:])
        nc.sync.dma_start(r1o[:], r1acc[:])

    nc.finalize()
    return nc


def preprocess(preds, targ, ncores=NCORES):
    """Host-side: slice/transposes/norms -> per-core input maps."""
    preds = np.asarray(preds)
    targ = np.asarray(targ)
    assert preds.shape == (B, H, T), preds.shape
    if preds.dtype != np.float32:
        preds = preds.astype(np.float32)
    if targ.dtype != np.float32:
        targ = targ.astype(np.float32)

    obs_p = preds[:, :, AD:].reshape(B, H, NPTS, D)
    obs_t = targ[:, :, AD:].reshape(B, H, NPTS, D)
    p_bf = obs_p.astype(BF16)           # y (preds)
    t_bf = obs_t.astype(BF16)           # x (targ / gts)
    # norms computed from the bf16-rounded values, in fp32
    y2 = np.square(p_bf.astype(np.float32)).sum(-1)   # [B, H, 256]
    x2 = np.square(t_bf.astype(np.float32)).sum(-1)

    act_p = preds[:, :, :AD].reshape(B * H, AD)
    act_t = targ[:, :, :AD].reshape(B * H, AD)

    bloc = B // ncores
    g = bloc * H
    in_maps = []
    for c in range(ncores):
        sl = slice(bloc * c, bloc * (c + 1))
        xt_c = np.ascontiguousarray(
            t_bf[sl].transpose(3, 0, 1, 2).reshape(D, g, NPTS))
        yt_c = np.ascontiguousarray(
            p_bf[sl].transpose(3, 0, 1, 2).reshape(D, g, NPTS))
        y2_c = (-0.5 * y2[sl].reshape(g, NPTS)).astype(BF16)
        y2d_c = np.concatenate([y2_c, y2_c], axis=1)          # [g, 512]
        x2_c = (-0.5 * x2[sl].reshape(g, NPTS)).astype(np.float32)
        x2b_c = np.ascontiguousarray(
            x2_c.reshape(g, 2, 128).transpose(2, 1, 0).reshape(128, 2 * g))
        rows = slice(g * c, g * (c + 1))
        ap_c = np.ascontiguousarray(
            act_p[rows].reshape(2, 128, AD).transpose(1, 0, 2).reshape(128, 8))
        at_c = np.ascontiguousarray(
            act_t[rows].reshape(2, 128, AD).transpose(1, 0, 2).reshape(128, 8))
        in_maps.append(dict(xt=xt_c, yt=yt_c, y2d=y2d_c, x2b=x2b_c,
                            actp=ap_c, actt=at_c))
    return in_maps


def postprocess(results):
    """Host-side: combine per-core partial outputs into the two loss scalars."""
    loss12 = 0.0
    mse = np.zeros((B, H), dtype=np.float64)
    bloc = B // len(results)
    for c, r in enumerate(results):
        loss12 += -2.0 * (r["r2o"].astype(np.float64).sum()
                          + r["r1o"].astype(np.float64).sum())
        aco = r["acto"].astype(np.float64)            # [128, 2]
        rows = aco.T.reshape(2 * 128) / AD            # row = c2*128 + p
        mse[bloc * c : bloc * (c + 1)] = rows.reshape(bloc, H)
    chamfer = loss12 / (B * H)
    a0_loss = mse[:, 0].mean()
    w = np.ones(H, dtype=np.float64)
    w[0] = 10.0
    action_loss = (mse * w[None, :]).mean()
    return (np.float32(action_loss + chamfer), np.float32(a0_loss))


_NC_CACHE = {}


def _get_program():
    if "nc" not in _NC_CACHE:
        _NC_CACHE["nc"] = build_program()
    return _NC_CACHE["nc"]


def kernel(preds, targ):
    nc = _get_program()
    in_maps = preprocess(preds, targ)
    results = run_bass_kernel_spmd(nc, in_maps, core_ids=list(range(NCORES))).results
    return postprocess(results)
